# revision 19
# baseline (speedup 1.0000x reference)
"""Multi-head attention (RoPE, causal) Trainium2 Bass kernel.

Problem: x[2,2048,1024], Wqkv[3072,1024], Wproj[1024,1024], H=16 heads, D=64.
Sharding: 8 cores = (2 batches) x (4 head-groups of 4 heads).  Each core
computes qkv + rope + causal attention + its partial output projection for its
4 heads; the host sums the 4 partial projections per batch.

Layout strategy (all matmuls in float32r, full PE rate at N>=256):
  - Host passes x transposed (xT [C,T]) so Q,K are produced directly in
    [d, t] layout (lhsT=W.T, rhs=xT) and V in natural [t, d] layout
    (lhsT=xT, rhs=Wv.T).  No on-chip transposes anywhere.
  - Attention runs in "S-transposed" orientation: S_T[k_pos, q] = K.T @ Q,
    2 heads row-packed per matmul pair (K=64 contraction each).
  - Softmax: no max-subtraction (logits are O(5)); exp on ScalarE with
    scale=1/8 folded in; causal band masked by adding -3e4 on the diagonal
    128x128 blocks; fully-masked columns excluded via matmul subranges.
  - P@V accumulates out_T[d, q] with lhsT=V (natural layout); denominator
    via a ones-column in V with M=1 matmuls into separate PSUM banks.
  - Projection: lhsT=ctx_T (stationary), rhs=Wproj slice -> natural [t, o]
    partial output, DMA'd out; host reduces.
"""

import numpy as np

import concourse.bass as bass
import concourse.mybir as mybir
import concourse.tile as tile

F32 = mybir.dt.float32
F32R = mybir.dt.float32r
EXP = mybir.ActivationFunctionType.Exp

B, T, C, H, D = 2, 2048, 1024, 16, 64
HL = H // 4          # 4 heads per core
N_CORES = 8
ROPE_BASE = 10000.0
SCALE = float(D) ** -0.5
NEG = -30000.0
TT = 512             # t-tile / q-tile size
NTT = T // TT        # 4
KB = 128             # k block
NKB = T // KB        # 16


# ---------------------------------------------------------------- legalizer
_wfx = [0]


def _legalize_sync_waits(nc, limit=1):
    """walrus in this container accepts only `limit` sync-waits per
    instruction; move excess waits onto preceding same-engine NOPs."""
    n_fixed = 0
    for f in nc.m.functions:
        for blk in f.blocks:
            insts = blk.instructions
            new_list = []
            changed = False
            for inst in insts:
                si = inst.sync_info
                if si is not None and len(si.on_wait) > limit:
                    waits = list(si.on_wait)
                    keep = waits[-limit:]
                    excess = waits[:-limit]
                    for k in range(0, len(excess), limit):
                        _wfx[0] += 1
                        new_list.append(mybir.InstNoOp(
                            name=f"waitfix_{_wfx[0]}",
                            engine=inst.engine,
                            bass_nofuse=True,
                            sync_info=mybir.SyncInfo(
                                on_wait=excess[k:k + limit], on_update=[]),
                        ))
                    si.on_wait = keep
                    changed = True
                    n_fixed += 1
                new_list.append(inst)
            if changed:
                insts.clear()
                insts.extend(new_list)
    return n_fixed


# ---------------------------------------------------------------- bass build
def build_bass():
    nc = bass.Bass("TRN2")
    xT = nc.dram_tensor("xT", [C, T], F32R, kind="ExternalInput")
    wqkT = nc.dram_tensor("wqkT", [C, 2 * HL * D], F32R, kind="ExternalInput")
    wvT = nc.dram_tensor("wvT", [C, HL * D], F32R, kind="ExternalInput")
    wpT = nc.dram_tensor("wpT", [HL * D, C], F32R, kind="ExternalInput")
    cosF = nc.dram_tensor("cosF", [128, T], F32, kind="ExternalInput")
    sinF = nc.dram_tensor("sinF", [128, T], F32, kind="ExternalInput")
    maskband = nc.dram_tensor("maskband", [128, 128], F32, kind="ExternalInput")
    vones = nc.dram_tensor("vones", [1, NKB * HL], F32R, kind="ExternalInput")
    y = nc.dram_tensor("y", [T, C], F32, kind="ExternalOutput")

    with tile.TileContext(nc) as tc:
        with tc.tile_pool(name="persist", bufs=1) as persist:
            # weights + tables (loaded once)
            wqk_sb = persist.tile([128, 8, 512], F32R, tag="wqk")
            nc.sync.dma_start(out=wqk_sb,
                              in_=wqkT.rearrange("(co p) o -> p co o", p=128))
            wv_sb = persist.tile([128, 8, 256], F32R, tag="wv")
            nc.sync.dma_start(out=wv_sb,
                              in_=wvT.rearrange("(co p) o -> p co o", p=128))
            wp_sb = persist.tile([128, 2, 1024], F32R, tag="wp")
            nc.sync.dma_start(out=wp_sb,
                              in_=wpT.rearrange("(kb p) o -> p kb o", p=128))
            cos_sb = persist.tile([128, T], F32, tag="cos")
            nc.sync.dma_start(out=cos_sb, in_=cosF[:, :])
            sin_sb = persist.tile([128, T], F32, tag="sin")
            nc.sync.dma_start(out=sin_sb, in_=sinF[:, :])
            mask_sb = persist.tile([128, 128], F32, tag="mask")
            nc.sync.dma_start(out=mask_sb, in_=maskband[:, :])

            # persistent activations
            qt_sb = [persist.tile([128, T], F32R, tag=f"qt{i}", name=f"qt{i}") for i in range(2)]
            kt_sb = [persist.tile([128, T], F32R, tag=f"kt{i}", name=f"kt{i}") for i in range(2)]
            ctx_sb = [persist.tile([128, T], F32R, tag=f"ctx{i}", name=f"ctx{i}") for i in range(2)]
            # V in natural layout + trailing ones column (the 65th matmul
            # output row becomes the softmax denominator): [t, kblk, head, 65]
            v_sb = persist.tile([128, NKB, HL, D + 1], F32R, tag="v")
            nc.sync.dma_start(out=v_sb[:, :, :, D:D + 1],
                              in_=vones[0:1, :].partition_broadcast(128))

            # ---------------- Phase B: QKV + RoPE ----------------
            with tc.tile_pool(name="xpool", bufs=2) as xpool, \
                 tc.tile_pool(name="ropetmp", bufs=3) as rpool, \
                 tc.tile_pool(name="qkps", bufs=4, space="PSUM") as qkps, \
                 tc.tile_pool(name="vps", bufs=2, space="PSUM") as vps:
                for tt in range(NTT):
                    ts = slice(tt * TT, (tt + 1) * TT)
                    x_sb = xpool.tile([128, 8, TT], F32R, tag="x")
                    nc.sync.dma_start(
                        out=x_sb,
                        in_=xT.rearrange("(co p) t -> p co t", p=128)[:, :, ts])
                    # Q (ob 0,1) and K (ob 2,3), pair-stacked 2 heads/tile
                    for ob in range(4):
                        qk_ps = qkps.tile([128, TT], F32, tag="qk")
                        for c in range(8):
                            nc.tensor.matmul(
                                qk_ps[:, :],
                                wqk_sb[:, c, ob * 128:(ob + 1) * 128],
                                x_sb[:, c, :],
                                start=(c == 0), stop=(c == 7))
                        dst = (qt_sb if ob < 2 else kt_sb)[ob % 2]
                        # rope: dst = raw*cos + shift32(raw)*sin_signed
                        raw = rpool.tile([128, TT], F32, tag="raw")
                        nc.scalar.copy(raw[:, :], qk_ps[:, :])
                        qc = rpool.tile([128, TT], F32, tag="qc")
                        nc.vector.tensor_mul(qc[:, :], raw[:, :], cos_sb[:, ts])
                        tmp = rpool.tile([128, TT], F32, tag="tmp")
                        for h2 in range(2):
                            b0 = h2 * 64
                            nc.sync.dma_start(out=tmp[b0:b0 + 32, :],
                                              in_=raw[b0 + 32:b0 + 64, :])
                            nc.sync.dma_start(out=tmp[b0 + 32:b0 + 64, :],
                                              in_=raw[b0:b0 + 32, :])
                        qs = rpool.tile([128, TT], F32, tag="qs")
                        nc.vector.tensor_mul(qs[:, :], tmp[:, :], sin_sb[:, ts])
                        nc.vector.tensor_add(dst[:, ts], qc[:, :], qs[:, :])
                    # V natural layout
                    for tb in range(4):
                        tg = tt * 4 + tb
                        v_ps = vps.tile([128, HL * D], F32, tag="vps")
                        for c in range(8):
                            nc.tensor.matmul(
                                v_ps[:, :],
                                x_sb[:, c, tb * 128:(tb + 1) * 128],
                                wv_sb[:, c, :],
                                start=(c == 0), stop=(c == 7))
                        nc.scalar.copy(
                            v_sb[:, tg, :, 0:D],
                            v_ps[:, :].rearrange("p (h d) -> p h d", d=D))

            # ---------------- Phase C: attention ----------------
            with tc.tile_pool(name="spool", bufs=1, space="PSUM") as spool, \
                 tc.tile_pool(name="opool", bufs=2, space="PSUM") as opool, \
                 tc.tile_pool(name="ptpool", bufs=2) as ptpool, \
                 tc.tile_pool(name="dramp", bufs=2, space="DRAM") as dramp, \
                 tc.tile_pool(name="npool", bufs=2) as npool:
                for p in range(2):
                    qt, kt = qt_sb[p], kt_sb[p]
                    hA, hB = 2 * p, 2 * p + 1
                    # per-pair denominator staging; engine copies need
                    # 32-granular start partitions, so qtile qi lands in row
                    # 32*qi.  memset 1.0 keeps the unused rows' reciprocals
                    # finite.
                    den_A = npool.tile([97, TT], F32, tag="denA", name="den_A")
                    den_B = npool.tile([97, TT], F32, tag="denB", name="den_B")
                    nc.vector.memset(den_A[:, :], 1.0)
                    nc.vector.memset(den_B[:, :], 1.0)
                    for qi in range(NTT):
                        qsl = slice(qi * TT, (qi + 1) * TT)
                        nkb = 4 * (qi + 1)
                        # rows 0-63: head ctx; row 64: denominator (ones col)
                        ctxA = opool.tile([65, TT], F32, tag="ctxA")
                        ctxB = opool.tile([65, TT], F32, tag="ctxB")
                        for g in range(nkb // 2):
                            sA = spool.tile([128, 2 * TT], F32, tag="sA")
                            sB = spool.tile([128, 2 * TT], F32, tag="sB")
                            ptA = ptpool.tile([128, 2 * TT], F32R, tag="ptA")
                            ptB = ptpool.tile([128, 2 * TT], F32R, tag="ptB")
                            js = (2 * g, 2 * g + 1)
                            for m, j in enumerate(js):
                                ksl = slice(j * KB, (j + 1) * KB)
                                msl = slice(m * TT, (m + 1) * TT)
                                nc.tensor.matmul(
                                    sA[:, msl], kt[0:64, ksl], qt[0:64, qsl],
                                    start=True, stop=True, tile_position=(0, 0))
                                nc.tensor.matmul(
                                    sB[:, msl], kt[64:128, ksl], qt[64:128, qsl],
                                    start=True, stop=True, tile_position=(64, 0))
                                delta = j * KB - qi * TT
                                if 0 <= delta < TT:
                                    bs = slice(m * TT + delta, m * TT + delta + KB)
                                    nc.vector.tensor_add(sA[:, bs], sA[:, bs],
                                                         mask_sb[:, :])
                                    nc.vector.tensor_add(sB[:, bs], sB[:, bs],
                                                         mask_sb[:, :])
                            nc.scalar.activation(ptA[:, :], sA[:, :], EXP,
                                                 scale=SCALE)
                            nc.scalar.activation(ptB[:, :], sB[:, :], EXP,
                                                 scale=SCALE)
                            for m, j in enumerate(js):
                                delta = j * KB - qi * TT
                                d0 = max(0, delta)
                                st, sp = (j == 0), (j == nkb - 1)
                                sub = slice(d0, TT)
                                rsub = slice(m * TT + d0, (m + 1) * TT)
                                nc.tensor.matmul(
                                    ctxA[0:65, sub], v_sb[:, j, hA, 0:D + 1],
                                    ptA[:, rsub], start=st, stop=sp)
                                nc.tensor.matmul(
                                    ctxB[0:65, sub], v_sb[:, j, hB, 0:D + 1],
                                    ptB[:, rsub], start=st, stop=sp)
                        # evacuate unnormalized ctx + denominators (partition-
                        # shifted engine copies are legal on TRN2)
                        nc.vector.tensor_copy(ctx_sb[p][0:64, qsl], ctxA[0:64, :])
                        nc.scalar.copy(ctx_sb[p][64:128, qsl], ctxB[0:64, :])
                        nc.scalar.copy(den_A[32 * qi:32 * qi + 1, :],
                                       ctxA[64:65, :])
                        nc.scalar.copy(den_B[32 * qi:32 * qi + 1, :],
                                       ctxB[64:65, :])

                    # ---- end of pair: reciprocal + broadcast + normalize ----
                    recA = npool.tile([97, TT], F32, tag="recA")
                    nc.vector.reciprocal(recA[:, :], den_A[:, :])
                    recB = npool.tile([97, TT], F32, tag="recB")
                    nc.vector.reciprocal(recB[:, :], den_B[:, :])
                    # SBUF APs cannot have partition step 0, so bounce the
                    # recip rows through DRAM and broadcast-read back.
                    recd = dramp.tile([8, TT], F32, tag="recd")
                    for qi in range(NTT):
                        nc.sync.dma_start(out=recd[qi:qi + 1, :],
                                          in_=recA[32 * qi:32 * qi + 1, :])
                        nc.sync.dma_start(out=recd[4 + qi:5 + qi, :],
                                          in_=recB[32 * qi:32 * qi + 1, :])
                    rb = npool.tile([128, T], F32, tag="rb")
                    for qi in range(NTT):
                        qsl = slice(qi * TT, (qi + 1) * TT)
                        nc.sync.dma_start(
                            out=rb[0:64, qsl],
                            in_=recd[qi:qi + 1, :].partition_broadcast(64))
                        nc.sync.dma_start(
                            out=rb[64:128, qsl],
                            in_=recd[4 + qi:5 + qi, :].partition_broadcast(64))
                    nc.vector.tensor_mul(ctx_sb[p][:, :], ctx_sb[p][:, :],
                                         rb[:, :])

            # ---------------- Phase D: output projection ----------------
            with tc.tile_pool(name="yps", bufs=2, space="PSUM") as yps, \
                 tc.tile_pool(name="ysb", bufs=3) as ysb:
                for tb in range(NKB):
                    tsl = slice(tb * 128, (tb + 1) * 128)
                    y_ps = yps.tile([128, 1024], F32, tag="y_ps")
                    for no in range(2):
                        osl = slice(no * 512, (no + 1) * 512)
                        for kb in range(2):
                            nc.tensor.matmul(
                                y_ps[:, osl], ctx_sb[kb][:, tsl],
                                wp_sb[:, kb, osl],
                                start=(kb == 0), stop=(kb == 1))
                    y_sb = ysb.tile([128, 1024], F32, tag="y_sb")
                    nc.vector.tensor_copy(y_sb[:, 0:512], y_ps[:, 0:512])
                    nc.scalar.copy(y_sb[:, 512:1024], y_ps[:, 512:1024])
                    nc.sync.dma_start(out=y[tsl, :], in_=y_sb[:, :])

    return nc


# ---------------------------------------------------------------- host side
def _rope_tables():
    inv = 1.0 / (ROPE_BASE ** (np.arange(0, D, 2, dtype=np.float32) / D))  # [32]
    t = np.arange(T, dtype=np.float32)
    freqs = np.outer(t, inv)                      # [T, 32]
    cosF = np.empty((128, T), dtype=np.float32)
    sinF = np.empty((128, T), dtype=np.float32)
    for p_ in range(128):
        d = p_ % D
        cosF[p_] = np.cos(freqs[:, d % 32])
        s = np.sin(freqs[:, d % 32])
        sinF[p_] = -s if d < 32 else s
    return cosF, sinF


def _mask_band():
    jp = np.arange(128)[:, None]
    qb = np.arange(128)[None, :]
    return np.where(qb >= jp, 0.0, NEG).astype(np.float32)


def prepare_in_maps(x, Wqkv, Wproj):
    x = np.asarray(x, dtype=np.float32)
    Wqkv = np.asarray(Wqkv, dtype=np.float32)
    Wproj = np.asarray(Wproj, dtype=np.float32)
    cosF, sinF = _rope_tables()
    mb = _mask_band()
    xTs = [np.ascontiguousarray(x[b].T) for b in range(B)]
    in_maps = []
    for core in range(N_CORES):
        b, g = divmod(core, 4)
        hs = [4 * g + i for i in range(HL)]
        q_rows = np.concatenate([Wqkv[h * D:(h + 1) * D] for h in hs])
        k_rows = np.concatenate([Wqkv[C + h * D:C + (h + 1) * D] for h in hs])
        v_rows = np.concatenate([Wqkv[2 * C + h * D:2 * C + (h + 1) * D] for h in hs])
        wqkT = np.ascontiguousarray(np.concatenate([q_rows, k_rows]).T)
        wvT = np.ascontiguousarray(v_rows.T)
        cols = np.concatenate([np.arange(h * D, (h + 1) * D) for h in hs])
        wpT = np.ascontiguousarray(Wproj[:, cols].T)
        in_maps.append(dict(xT=xTs[b], wqkT=wqkT, wvT=wvT, wpT=wpT,
                            cosF=cosF, sinF=sinF, maskband=mb,
                            vones=np.ones((1, NKB * HL), dtype=np.float32)))
    return in_maps


_CACHE = {}


def _get_nc():
    if "nc" not in _CACHE:
        _CACHE["nc"] = build_bass()
    return _CACHE["nc"]


def _run(in_maps):
    from concourse.bass_utils import run_bass_kernel_spmd
    nc = _get_nc()
    if not _CACHE.get("legalized"):
        _legalize_sync_waits(nc, limit=1)
        _CACHE["legalized"] = True
    res = run_bass_kernel_spmd(nc, in_maps, core_ids=list(range(N_CORES)))
    return res.results


def kernel(x, Wqkv, Wproj):
    in_maps = prepare_in_maps(x, Wqkv, Wproj)
    results = _run(in_maps)
    out = np.zeros((B, T, C), dtype=np.float32)
    for core in range(N_CORES):
        b = core // 4
        out[b] += results[core]["y"]
    return out
